# revision 1
# baseline (speedup 1.0000x reference)
"""Multi-head attention block on 8 Trainium2 NeuronCores.

Problem: x[8,1024,768] -> qkv = x@w_qkv+b_qkv -> 12-head attention -> proj.
Sharding: pure data-parallel over batch (B=8 -> 1 batch element per core).
No collectives needed.

Per-core design (tokens n=1024, features d=768, heads h=12, hd=64):
  - x^T [768,1024] via PE transpose (fp32 has no DMA transpose)
  - v = x @ w_qkv[:,1536:] + b in natural [token, feature] layout
    (x^T slices as the stationary operand), stored fp16 with an extra
    ones column per (m-tile, head) -> [v | 1]
  - per head-pair hp (heads 2hp / 2hp+1):
      q^T,k^T f-tiles hp and 6+hp of (x @ w_qkv)^T: w_qkv tiles as
      stationary, x^T moving; per-head slices are [64, 1024] at
      partition base (h%2)*64
      scores^T[m,n] = k^T-slice.T @ q^T-slice: K=64 matmuls, the two
      heads alternate PE row groups 0/64 and run concurrently
      P = exp(scores/8) on ACT (no max subtraction: |scores| < ~8), fp16
      attnv: out^T rows 0..63 + softmax-denominator row 64 via [v | 1]
      normalize: DVE reciprocal + gpsimd partition_broadcast + DVE mult
    The pair loop is software-pipelined: pair hp's qk/scores run on PE
    while pair hp-1's attnv waits for its exps on ACT (ACT is the pacing
    engine in steady state).
  - proj: wa^T slices stationary, w_proj moving -> out in natural [n, d]
    layout, no final transpose.
Matmuls run as float32r (full PE rate, walrus requires producers to
round to f32r); P/v in fp16.
"""

import numpy as np

import concourse.bass as bass
import concourse.mybir as mybir
from concourse import bacc
from concourse.tile import TileContext
from concourse.bass_utils import run_bass_kernel_spmd
from concourse.masks import make_identity

P = 128
N = 1024          # tokens per batch element
D = 768           # model dim
H = 12            # heads
HD = 64           # head dim
KT = D // P       # 6 k-tiles over model dim
NT = N // P       # 8 token tiles
NCORES = 8
SCALE = HD ** -0.5  # 0.125

F32 = mybir.dt.float32
F32R = mybir.dt.float32r
BF16 = mybir.dt.bfloat16
F16 = mybir.dt.float16


def _emit(nc, reps=1):
    x = nc.dram_tensor("x", [N, D], F32, kind="ExternalInput")
    w_qkv = nc.dram_tensor("w_qkv", [D, 3 * D], F32, kind="ExternalInput")
    b_qkv = nc.dram_tensor("b_qkv", [3 * D], F32, kind="ExternalInput")
    w_proj = nc.dram_tensor("w_proj", [D, D], F32, kind="ExternalInput")
    b_proj = nc.dram_tensor("b_proj", [D], F32, kind="ExternalInput")
    out = nc.dram_tensor("out", [N, D], F32, kind="ExternalOutput")

    with TileContext(nc) as tc:
      for _rep in range(reps):
        with tc.tile_pool(name="main", bufs=1) as main, \
             tc.tile_pool(name="outbuf", bufs=2) as outbuf:
            v_sb = main.tile([P, NT, H, HD + 1], F16)   # v + ones column
            wa_sb = main.tile([P, KT, N], F32R)          # normalized attn out ^T
            bq_sb = main.tile([P, 2 * KT], F32)          # q,k bias (per partition)
            vb_sb = main.tile([P, D], F32)               # v bias (bcast over partitions)
            pb_sb = main.tile([P, D], F32)               # proj bias (bcast)
            ident = main.tile([P, P], F32)

            make_identity(nc, ident[:])
            nc.gpsimd.memset(v_sb[:, :, :, HD:HD + 1], 1.0)
            nc.gpsimd.dma_start(bq_sb[:], b_qkv[0:2 * D].rearrange("(o p) -> p o", p=P))
            nc.gpsimd.dma_start(vb_sb[:], b_qkv[2 * D:3 * D].unsqueeze(0).partition_broadcast(P))
            nc.gpsimd.dma_start(pb_sb[:], b_proj[:].unsqueeze(0).partition_broadcast(P))

            with tc.tile_pool(name="xt", bufs=1) as xtp, \
                 tc.tile_pool(name="wp", bufs=1) as wpp:
                xT = xtp.tile([P, KT, N], F32R)
                wp_sb = wpp.tile([P, KT, D], F32R)
                for kt in range(KT):
                    nc.gpsimd.dma_start(wp_sb[:, kt, :], w_proj[kt * P:(kt + 1) * P, :].bitcast(F32R))

                # ---- Phase A: load x, PE-transpose to x^T ----
                with tc.tile_pool(name="xload", bufs=3) as xlp, \
                     tc.tile_pool(name="tpsum", bufs=4, space="PSUM") as tpp:
                    for nt in range(NT):
                        xt = xlp.tile([P, D], F32)
                        nc.sync.dma_start(xt[:], x[nt * P:(nt + 1) * P, :])
                        for kt in range(KT):
                            pst = tpp.tile([P, P], F32, tag="tp", name=f"tp_{nt}_{kt}")
                            nc.tensor.transpose(pst[:], xt[:, kt * P:(kt + 1) * P], ident[:])
                            nc.vector.tensor_copy(xT[:, kt, nt * P:(nt + 1) * P], pst[:])

                # ---- Phase C: qk + attention, pipelined over head pairs ----
                with tc.tile_pool(name="wqk", bufs=12) as wqkp, \
                     tc.tile_pool(name="wv", bufs=1) as wvp, \
                     tc.tile_pool(name="qk", bufs=2) as qkp, \
                     tc.tile_pool(name="p", bufs=3) as ppool, \
                     tc.tile_pool(name="stat", bufs=2) as statp, \
                     tc.tile_pool(name="qkpsum", bufs=1, space="PSUM") as qpp, \
                     tc.tile_pool(name="spsum", bufs=2, space="PSUM") as spp, \
                     tc.tile_pool(name="opsum", bufs=2, space="PSUM") as opp:

                    wv_sb = wvp.tile([P, KT, D], F32R)
                    for kt in range(KT):
                        nc.gpsimd.dma_start(wv_sb[:, kt, :], w_qkv[kt * P:(kt + 1) * P, 2 * D:3 * D].bitcast(F32R))

                    def emit_v():
                        # v projection in natural [token, feature] layout;
                        # shares the opsum pool slots (runs before any attnv)
                        for nt in range(NT):
                            for c2 in range(2):
                                fs = slice(c2 * 384, (c2 + 1) * 384)
                                psv = opp.tile([P, 384], F32, tag="opsum", name=f"vpsum_{nt}_{c2}")
                                for kt in range(KT):
                                    nc.tensor.matmul(psv[:], xT[:, kt, nt * P:(nt + 1) * P],
                                                     wv_sb[:, kt, fs],
                                                     start=(kt == 0), stop=(kt == KT - 1))
                                nc.vector.tensor_add(
                                    v_sb[:, nt, c2 * 6:(c2 + 1) * 6, 0:HD],
                                    psv[:].rearrange("p (h d) -> p h d", d=HD),
                                    vb_sb[:, fs].rearrange("p (h d) -> p h d", d=HD))

                    def emit_qk(hp):
                        # q^T f-tile hp and k^T f-tile 6+hp for this pair
                        qk_t = qkp.tile([P, 2, N], F32R, tag="qk", name=f"qk_{hp}")
                        for i, ft in enumerate((hp, 6 + hp)):
                            wts = []
                            for kt in range(KT):
                                wt = wqkp.tile([P, P], F32R, tag="wqk", name=f"w_{ft}_{kt}")
                                nc.sync.dma_start(wt[:], w_qkv[kt * P:(kt + 1) * P, ft * P:(ft + 1) * P].bitcast(F32R))
                                wts.append(wt)
                            ps = qpp.tile([P, N], F32, tag="qkpsum", name=f"qkps_{ft}")
                            for ch in range(2):
                                cs = slice(ch * 512, (ch + 1) * 512)
                                for kt in range(KT):
                                    nc.tensor.matmul(ps[:, cs], wts[kt][:], xT[:, kt, cs],
                                                     start=(kt == 0), stop=(kt == KT - 1))
                            nc.vector.tensor_add(qk_t[:, i, :], ps[:],
                                                 bq_sb[:, ft:ft + 1].to_broadcast([P, N]))
                        return qk_t

                    def emit_scores(hp, qk_t, ptiles):
                        # the two heads alternate PE row groups 0/64
                        heads = (2 * hp, 2 * hp + 1)
                        for mt in range(NT):
                            spss = {
                                h: spp.tile([P, N], F32, tag="spsum", name=f"spsum_{h}_{mt}")
                                for h in heads
                            }
                            for ch in range(2):
                                cs = slice(ch * 512, (ch + 1) * 512)
                                for h in heads:
                                    base = (h % 2) * HD
                                    nc.tensor.matmul(
                                        spss[h][:, cs],
                                        qk_t[base:base + HD, 1, mt * P:(mt + 1) * P],
                                        qk_t[base:base + HD, 0, cs],
                                        start=True, stop=True)
                            for h in heads:
                                nc.scalar.activation(ptiles[h][:, mt, :], spss[h][:],
                                                     mybir.ActivationFunctionType.Exp,
                                                     scale=SCALE)

                    def emit_attnv(hp, ptiles):
                        for h in (2 * hp, 2 * hp + 1):
                            base = (h % 2) * HD
                            psos = []
                            s0 = statp.tile([1, N], F32, tag="s0", name=f"s0_{h}")
                            rb = statp.tile([HD, N], F32, tag="rb", name=f"rb_{h}")
                            for ch in range(2):
                                cs = slice(ch * 512, (ch + 1) * 512)
                                pso = opp.tile([HD + 1, 512], F32, tag="opsum", name=f"opsum_{h}_{ch}")
                                for mt in range(NT):
                                    nc.tensor.matmul(pso[:], v_sb[:, mt, h, :],
                                                     ptiles[h][:, mt, cs],
                                                     start=(mt == 0), stop=(mt == NT - 1))
                                nc.vector.tensor_copy(s0[0:1, cs], pso[HD:HD + 1, :])
                                psos.append(pso)
                            nc.vector.reciprocal(s0[:], s0[:])
                            nc.gpsimd.partition_broadcast(rb[:], s0[:])
                            for ch in range(2):
                                cs = slice(ch * 512, (ch + 1) * 512)
                                nc.vector.tensor_mul(wa_sb[base:base + HD, h // 2, cs],
                                                     psos[ch][0:HD, :], rb[:, cs])

                    prev = None
                    for hp in range(H // 2):
                        qk_t = emit_qk(hp)
                        ptiles = {
                            h: ppool.tile([P, NT, N], F16, tag="p", name=f"p_{h}")
                            for h in (2 * hp, 2 * hp + 1)
                        }
                        emit_scores(hp, qk_t, ptiles)
                        if hp == 0:
                            # v projection overlaps pair 0's exps on ACT
                            emit_v()
                        if prev is not None:
                            emit_attnv(prev[0], prev[1])
                        prev = (hp, ptiles)
                    emit_attnv(prev[0], prev[1])

                # ---- Phase D: output projection ----
                with tc.tile_pool(name="prpsum", bufs=4, space="PSUM") as prp:
                    for nt in range(NT):
                        ot = outbuf.tile([P, D], F32, tag="out", name=f"out_{nt}")
                        for jc in range(2):
                            js = slice(jc * 384, (jc + 1) * 384)
                            psp = prp.tile([P, 384], F32, tag="prpsum", name=f"prps_{nt}_{jc}")
                            for kt in range(KT):
                                nc.tensor.matmul(psp[:], wa_sb[:, kt, nt * P:(nt + 1) * P],
                                                 wp_sb[:, kt, js],
                                                 start=(kt == 0), stop=(kt == KT - 1))
                            nc.vector.tensor_add(ot[:, js], psp[:], pb_sb[:, js])
                        nc.sync.dma_start(out[nt * P:(nt + 1) * P, :], ot[:])


def build(reps=1):
    nc = bacc.Bacc("TRN2", target_bir_lowering=False, debug=False, num_devices=NCORES)
    _emit(nc, reps=reps)
    nc.compile()
    return nc


_CACHE = {}


def _get_nc():
    if "nc" not in _CACHE:
        _CACHE["nc"] = build()
    return _CACHE["nc"]


def kernel(x, w_qkv, b_qkv, w_proj, b_proj):
    x = np.ascontiguousarray(np.asarray(x, dtype=np.float32))
    w_qkv = np.ascontiguousarray(np.asarray(w_qkv, dtype=np.float32))
    b_qkv = np.ascontiguousarray(np.asarray(b_qkv, dtype=np.float32))
    w_proj = np.ascontiguousarray(np.asarray(w_proj, dtype=np.float32))
    b_proj = np.ascontiguousarray(np.asarray(b_proj, dtype=np.float32))

    nc = _get_nc()
    in_maps = [
        {"x": np.ascontiguousarray(x[c]), "w_qkv": w_qkv, "b_qkv": b_qkv,
         "w_proj": w_proj, "b_proj": b_proj}
        for c in range(NCORES)
    ]
    res = run_bass_kernel_spmd(nc, in_maps, list(range(NCORES)))
    return np.stack([res.results[c]["out"] for c in range(NCORES)], axis=0)



# revision 5
# speedup vs baseline: 2.7238x; 2.7238x over previous
"""Multi-head attention block on 8 Trainium2 NeuronCores.

Problem: x[8,1024,768] -> qkv = x@w_qkv+b_qkv -> 12-head attention -> proj.
Sharding: pure data-parallel over batch (B=8 -> 1 batch element per core).
No collectives needed.

Per-core design (tokens n=1024, features d=768, heads h=12, hd=64):
  - x^T [768,1024] via PE transpose (fp32 has no DMA transpose).
  - v = x @ w_qkv[:,1536:] + b in natural [token, feature] layout, stored
    fp16 with 64 ones columns per (m-tile, head): [v | 1...1]. The ones
    replicate the softmax denominator across PSUM rows 64..127 of the
    attnv output, so the normalize needs no partition broadcast and no
    single-partition ops.
  - per head-pair hp (heads 2hp / 2hp+1):
      q^T,k^T f-tiles hp and 6+hp of (x @ w_qkv)^T; per-head slices are
      [64, 1024] at partition base (h%2)*64
      scores^T[m,n] = k^T-slice.T @ q^T-slice (K=64, the two heads on PE
      row groups 0/64), both heads into one [128,2048] PSUM tile
      P = exp(scores/8) on ACT in ONE [128,2048] ACTIVATE per m-tile,
      fp16 (no max subtraction: |scores/8| < ~8)
      attnv: psum rows 0..63 = out^T rows, rows 64..127 = denominator
      (replicated); normalize = reciprocal_approx_fast + one fused
      multiply into wa
    Pipeline: qk runs one pair ahead of scores; attnv trails one pair.
    ACT (exp) is the pacing engine in the pair loop (~16us/pair); the PE
    fills stalls with the trailing attnv + leading qk.
  - proj: wa^T slices stationary, w_proj moving -> out in natural [n, d]
    layout.
Matmuls run as float32r (full PE rate); P/v in fp16.
PSUM budget: scores 1x[128,2048] (4 banks) + gen 2x[128,512] (2 banks,
shared by transpose/v/qk/proj) + attnv 2x[128,512] (2 banks) = 8 banks.
"""

import numpy as np

import concourse.bass as bass
import concourse.mybir as mybir
from concourse import bacc
from concourse.tile import TileContext
from concourse.bass_utils import run_bass_kernel_spmd
from concourse.masks import make_identity

P = 128
N = 1024          # tokens per batch element
D = 768           # model dim
H = 12            # heads
HD = 64           # head dim
KT = D // P       # 6 k-tiles over model dim
NT = N // P       # 8 token tiles
NP = H // 2       # 6 head pairs
NCORES = 8
SCALE = HD ** -0.5  # 0.125

F32 = mybir.dt.float32
F32R = mybir.dt.float32r
F16 = mybir.dt.float16


def _emit_rep(nc, tc, rep, ident, v_sb, x, w_qkv, b_qkv, w_proj, b_proj, out):
    with tc.tile_pool(name="sb", bufs=1) as sb, \
         tc.tile_pool(name="wbig", bufs=1) as wbig, \
         tc.tile_pool(name="xload", bufs=2) as xlp, \
         tc.tile_pool(name="wqk", bufs=24) as wqkp, \
         tc.tile_pool(name="qkt", bufs=2) as qktp, \
         tc.tile_pool(name="pp", bufs=2) as ppp, \
         tc.tile_pool(name="rcp", bufs=2) as rcpp, \
         tc.tile_pool(name="outb", bufs=2) as outbp, \
         tc.tile_pool(name="scps", bufs=1, space="PSUM") as scp, \
         tc.tile_pool(name="genps", bufs=2, space="PSUM") as genp, \
         tc.tile_pool(name="psop", bufs=2, space="PSUM") as psop:

        xT = sb.tile([P, KT, N], F32R, name=f"xT_{rep}")
        wa_sb = sb.tile([P, KT, N], F32R, name=f"wa_{rep}")
        bq_sb = sb.tile([P, 2 * KT], F32, name=f"bq_{rep}")
        vb_sb = sb.tile([P, D], F32, name=f"vb_{rep}")
        pb_sb = sb.tile([P, D], F32, name=f"pb_{rep}")

        nc.gpsimd.dma_start(bq_sb[:], b_qkv[0:2 * D].rearrange("(o p) -> p o", p=P))
        nc.gpsimd.dma_start(vb_sb[:], b_qkv[2 * D:3 * D].unsqueeze(0).partition_broadcast(P))
        nc.gpsimd.dma_start(pb_sb[:], b_proj[:].unsqueeze(0).partition_broadcast(P))

        wv_sb = wbig.tile([P, KT, D], F32R, tag="wbig", name=f"wv_{rep}")
        for kt in range(KT):
            nc.gpsimd.dma_start(wv_sb[:, kt, :], w_qkv[kt * P:(kt + 1) * P, 2 * D:3 * D].bitcast(F32R))

        def emit_qk(hp):
            qk_t = qktp.tile([P, 2, N], F32R, tag="qkt", name=f"qk_{rep}_{hp}")
            for i, ft in enumerate((hp, 6 + hp)):
                wts = []
                for kt in range(KT):
                    wt = wqkp.tile([P, P], F32R, tag="wqk", name=f"w_{rep}_{ft}_{kt}")
                    nc.sync.dma_start(wt[:], w_qkv[kt * P:(kt + 1) * P, ft * P:(ft + 1) * P].bitcast(F32R))
                    wts.append(wt)
                psq = [genp.tile([P, 512], F32, tag="gen", name=f"qkps_{rep}_{ft}_{ch}")
                       for ch in range(2)]
                for kt in range(KT):
                    for ch in range(2):
                        cs = slice(ch * 512, (ch + 1) * 512)
                        nc.tensor.matmul(psq[ch][:], wts[kt][:], xT[:, kt, cs],
                                         start=(kt == 0), stop=(kt == KT - 1))
                for ch in range(2):
                    cs = slice(ch * 512, (ch + 1) * 512)
                    nc.vector.tensor_add(qk_t[:, i, cs], psq[ch][:],
                                         bq_sb[:, ft:ft + 1].to_broadcast([P, 512]))
            return qk_t

        def emit_scores(hp, qk_t, pp):
            for mt in range(NT):
                ms = slice(mt * P, (mt + 1) * P)
                spt = scp.tile([P, 2, N], F32, tag="sc", name=f"sc_{rep}_{hp}_{mt}")
                for ch in range(2):
                    cs = slice(ch * 512, (ch + 1) * 512)
                    for i in range(2):
                        base = i * HD
                        nc.tensor.matmul(
                            spt[:, i, cs],
                            qk_t[base:base + HD, 1, ms],
                            qk_t[base:base + HD, 0, cs],
                            start=True, stop=True,
                            tile_position=(base, 0))
                nc.scalar.activation(pp[:, mt, :, :], spt[:, :, :],
                                     mybir.ActivationFunctionType.Exp,
                                     scale=SCALE)

        def emit_attnv(p, pp):
            # pair p: heads 2p (i=0), 2p+1 (i=1)
            for i in range(2):
                h = 2 * p + i
                base = i * HD
                pso = [psop.tile([P, 512], F32, tag="pso", name=f"pso_{rep}_{h}_{ch}")
                       for ch in range(2)]
                for mt in range(NT):
                    for ch in range(2):
                        cs = slice(ch * 512, (ch + 1) * 512)
                        nc.tensor.matmul(pso[ch][:], v_sb[:, mt, h, :],
                                         pp[:, mt, i, cs],
                                         start=(mt == 0), stop=(mt == NT - 1))
                for ch in range(2):
                    cs = slice(ch * 512, (ch + 1) * 512)
                    rcp = rcpp.tile([HD, 512], F32, tag="rcp", name=f"rcp_{rep}_{h}_{ch}")
                    nc.vector.reciprocal_approx_fast(rcp[:], pso[ch][0:HD, :])
                    nc.vector.tensor_mul(wa_sb[base:base + HD, p, cs],
                                         pso[ch][HD:P, :], rcp[:])

        # ---- Phase A: x load + transpose ----
        for nt in range(NT):
            xt = xlp.tile([P, D], F32, tag="xl", name=f"xl_{rep}_{nt}")
            nc.sync.dma_start(xt[:], x[nt * P:(nt + 1) * P, :])
            for kt in range(KT):
                pst = genp.tile([P, P], F32, tag="gen", name=f"tp_{rep}_{nt}_{kt}")
                nc.tensor.transpose(pst[:], xt[:, kt * P:(kt + 1) * P], ident[:])
                nc.vector.tensor_copy(xT[:, kt, nt * P:(nt + 1) * P], pst[:])

        # qk for pair 0 goes first so ACT (exp) starts as early as possible
        qk_prev = emit_qk(0)

        # ---- Phase B: v projection (natural layout, + bias, fp16) ----
        for nt in range(NT):
            psv = [genp.tile([P, 384], F32, tag="gen", name=f"vps_{rep}_{nt}_{c2}")
                   for c2 in range(2)]
            for kt in range(KT):
                for c2 in range(2):
                    fs = slice(c2 * 384, (c2 + 1) * 384)
                    nc.tensor.matmul(psv[c2][:], xT[:, kt, nt * P:(nt + 1) * P],
                                     wv_sb[:, kt, fs],
                                     start=(kt == 0), stop=(kt == KT - 1))
            for c2 in range(2):
                fs = slice(c2 * 384, (c2 + 1) * 384)
                nc.vector.tensor_add(
                    v_sb[:, nt, c2 * 6:(c2 + 1) * 6, HD:P],
                    psv[c2][:].rearrange("p (h d) -> p h d", d=HD),
                    vb_sb[:, fs].rearrange("p (h d) -> p h d", d=HD))

        # w_proj shares the wbig slot with wv (slot frees after v-proj; the
        # DMA hides under the pair loop)
        wp_sb = wbig.tile([P, KT, D], F32R, tag="wbig", name=f"wp_{rep}")
        for kt in range(KT):
            nc.gpsimd.dma_start(wp_sb[:, kt, :], w_proj[kt * P:(kt + 1) * P, :].bitcast(F32R))

        # ---- Phase C: pair loop, software-pipelined ----
        pp_prev = None
        for hp in range(NP):
            if hp + 1 < NP:
                qk_next = emit_qk(hp + 1)
            if pp_prev is not None:
                emit_attnv(hp - 1, pp_prev)
            pp = ppp.tile([P, NT, 2, N], F16, tag="pp", name=f"pp_{rep}_{hp}")
            emit_scores(hp, qk_prev, pp)
            pp_prev = pp
            if hp + 1 < NP:
                qk_prev = qk_next
        emit_attnv(NP - 1, pp_prev)

        # ---- Phase D: output projection ----
        for nt in range(NT):
            ns = slice(nt * P, (nt + 1) * P)
            ot = outbp.tile([P, D], F32, tag="out", name=f"out_{rep}_{nt}")
            psp = [genp.tile([P, 384], F32, tag="gen", name=f"prps_{rep}_{nt}_{jc}")
                   for jc in range(2)]
            for kt in range(KT):
                for jc in range(2):
                    js = slice(jc * 384, (jc + 1) * 384)
                    nc.tensor.matmul(psp[jc][:], wa_sb[:, kt, ns],
                                     wp_sb[:, kt, js],
                                     start=(kt == 0), stop=(kt == KT - 1))
            for jc in range(2):
                js = slice(jc * 384, (jc + 1) * 384)
                nc.vector.tensor_add(ot[:, js], psp[jc][:], pb_sb[:, js])
            nc.sync.dma_start(out[ns, :], ot[:])


def _emit(nc, reps=1):
    x = nc.dram_tensor("x", [N, D], F32, kind="ExternalInput")
    w_qkv = nc.dram_tensor("w_qkv", [D, 3 * D], F32, kind="ExternalInput")
    b_qkv = nc.dram_tensor("b_qkv", [3 * D], F32, kind="ExternalInput")
    w_proj = nc.dram_tensor("w_proj", [D, D], F32, kind="ExternalInput")
    b_proj = nc.dram_tensor("b_proj", [D], F32, kind="ExternalInput")
    out = nc.dram_tensor("out", [N, D], F32, kind="ExternalOutput")

    with TileContext(nc) as tc:
        with tc.tile_pool(name="const", bufs=1) as constp:
            ident = constp.tile([P, P], F32)
            v_sb = constp.tile([P, NT, H, P], F16)  # [...,0:64]=1, [...,64:128]=v
            make_identity(nc, ident[:])
            nc.gpsimd.memset(v_sb[:, :, :, 0:HD], 1.0)
            for rep in range(reps):
                _emit_rep(nc, tc, rep, ident, v_sb, x, w_qkv, b_qkv,
                          w_proj, b_proj, out)


def build(reps=1):
    nc = bacc.Bacc("TRN2", target_bir_lowering=False, debug=False, num_devices=NCORES)
    _emit(nc, reps=reps)
    nc.compile()
    return nc


_CACHE = {}


def _get_nc():
    if "nc" not in _CACHE:
        _CACHE["nc"] = build()
    return _CACHE["nc"]


def kernel(x, w_qkv, b_qkv, w_proj, b_proj):
    x = np.ascontiguousarray(np.asarray(x, dtype=np.float32))
    w_qkv = np.ascontiguousarray(np.asarray(w_qkv, dtype=np.float32))
    b_qkv = np.ascontiguousarray(np.asarray(b_qkv, dtype=np.float32))
    w_proj = np.ascontiguousarray(np.asarray(w_proj, dtype=np.float32))
    b_proj = np.ascontiguousarray(np.asarray(b_proj, dtype=np.float32))

    nc = _get_nc()
    in_maps = [
        {"x": np.ascontiguousarray(x[c]), "w_qkv": w_qkv, "b_qkv": b_qkv,
         "w_proj": w_proj, "b_proj": b_proj}
        for c in range(NCORES)
    ]
    res = run_bass_kernel_spmd(nc, in_maps, list(range(NCORES)))
    return np.stack([res.results[c]["out"] for c in range(NCORES)], axis=0)
